# revision 15
# baseline (speedup 1.0000x reference)
"""Trainium2 Bass kernel for BitLinear: y[b,s,o] = sum_d x[b,s,d] * w[o,d].

x: [4, 2048, 4096] f32, weight: [4096, 4096] int32 (values 0..255), y f32.

Strategy:
- Data-parallel over tokens: 8192 tokens -> 8 cores x 1024 tokens.
- Precision: SINGLE bf16 pass. Weight values 0..255 are exact in bf16
  (8-bit mantissa), so the only error is x's bf16 quantization (~2^-9
  rel): max rel err ~1.9e-3 against the 2e-2 gate, at half the matmul
  work of a hi/lo split. (fp8 DoubleRow was measured at 2x FLOP rate =
  157 TF/s, but the accuracy-required 3-term scheme needs 1.5x the
  bf16 single-pass time -> strictly worse.)
- W-stationary: yt[n, m] = W^T[k, n]^T @ X^T[k, m]; each 128-col
  stationary load feeds 2 moving M=512 matmuls (the two 512-token PSUM
  banks). 2048 MMs/core -> ~437 us back-to-back at 2.4 GHz; measured
  PE occupancy ~96% in TimelineSim.
- W is host-pre-tiled to [16, 128, 8192] so each output group's
  weights arrive in ONE contiguous 2MB DMA (16 weight DMAs per pass,
  not 512 -- per-DMA dispatch overhead dominated the un-batched
  version), double-buffered under the previous group's compute. The
  FIRST group of the first pass instead loads per-k 64KB slices so the
  PE starts ~2 us in rather than waiting for the full slab.
- X^T shard (8 MB bf16) streams in during group 0 and stays resident.
- Evictions: per 128-row slice, 2 PSUM banks -> one [128,1024] SBUF
  tile -> one DMA (64 vector copies + 32 output DMAs per pass).
- Host gathers per-core yt [4096, 1024] f32, transposes, concatenates.
"""

import sys

for _p in ("/opt/trn_rl_repo", "/root/.axon_site/_ro/trn_rl_repo"):
    if _p not in sys.path:
        sys.path.append(_p)

import numpy as np
import ml_dtypes

N_CORES = 8
TOKENS = 8192
D_IN = 4096
D_OUT = 4096
T_SHARD = TOKENS // N_CORES  # 1024
NG = D_OUT // 256  # 16 output groups

_NC_CACHE = {}


def build_nc(repeats: int = 1):
    """Build (and cache) the Bass program.

    repeats > 1 re-emits the compute body (used only for slope-based HW
    timing; identical output)."""
    if repeats in _NC_CACHE:
        return _NC_CACHE[repeats]

    import concourse.mybir as mybir
    import concourse.tile as tile
    from concourse import bacc

    P = 128
    KT = D_IN // P  # 32
    nc = bacc.Bacc(None, target_bir_lowering=False)
    with tile.TileContext(nc) as tc:
        with tc.tile_pool(name="dram", bufs=1, space="DRAM") as dram:
            kxm = dram.tile([D_IN, T_SHARD], mybir.dt.bfloat16,
                            kind="ExternalInput", name="kxm", uniquify=False)
            kxns = dram.tile([NG, P, KT * 256], mybir.dt.bfloat16,
                             kind="ExternalInput", name="kxns", uniquify=False)
            yt = dram.tile([D_OUT, T_SHARD], mybir.dt.float32,
                           kind="ExternalOutput", name="yt", uniquify=False)
            kxm3 = kxm[:].rearrange("(ko p) m -> p ko m", p=P)  # [128,32,1024]
            with tc.tile_pool(name="xpool", bufs=32) as xpool, \
                 tc.tile_pool(name="wpool", bufs=2) as wpool, \
                 tc.tile_pool(name="pspool", bufs=2, space="PSUM") as pspool, \
                 tc.tile_pool(name="evpool", bufs=4) as evpool:
                xtiles = [None] * KT
                first = True
                for rep in range(repeats):
                    for ng in range(NG):
                        wt = wpool.tile([P, KT * 256], mybir.dt.bfloat16,
                                        name="wt", tag="wt")
                        if rep == 0 and ng <= 1:
                            # cold start: groups 0-1 load W in chunks so
                            # their first matmuls wait on the first 128KB,
                            # not the full 2MB slab (the x stream owns most
                            # of the HBM window during group 0)
                            bounds = [0, 2, 8, 16, 24, 32]
                            for q in range(len(bounds) - 1):
                                nc.sync.dma_start(
                                    wt[:, bounds[q] * 256:bounds[q + 1] * 256],
                                    kxns[ng][:, bounds[q] * 256:
                                             bounds[q + 1] * 256])
                        else:
                            nc.sync.dma_start(wt[:], kxns[ng])
                        banks = {}
                        for nsl in range(2):
                            for mc in range(2):
                                banks[(nsl, mc)] = pspool.tile(
                                    [P, 512], mybir.dt.float32,
                                    name=f"bank_{nsl}_{mc}",
                                    tag=f"bank_{nsl}_{mc}")
                        if ng < NG - 1:
                            for k in range(KT):
                                if first:
                                    # JIT x tiles: X streaming hides under
                                    # group-0 compute and stays resident;
                                    # Activation-engine DGE queue so x
                                    # streams concurrently with W on SP's
                                    xt = xpool.tile([P, T_SHARD],
                                                    mybir.dt.bfloat16,
                                                    name="xt", tag="xt")
                                    nc.scalar.dma_start(xt[:], kxm3[:, k])
                                    xtiles[k] = xt
                                for nsl in range(2):
                                    lhsT = wt[:, k * 256 + nsl * P:
                                              k * 256 + (nsl + 1) * P]
                                    for mc in range(2):
                                        nc.tensor.matmul(
                                            banks[(nsl, mc)][:],
                                            lhsT,
                                            xtiles[k][
                                                :, mc * 512:(mc + 1) * 512],
                                            start=(k == 0),
                                            stop=(k == KT - 1),
                                        )
                            first = False
                            for nsl in range(2):
                                ev = evpool.tile(
                                    [P, T_SHARD], mybir.dt.float32,
                                    name="ev", tag="ev")
                                for mc in range(2):
                                    nc.vector.tensor_copy(
                                        out=ev[:, mc * 512:(mc + 1) * 512],
                                        in_=banks[(nsl, mc)][:])
                                nc.scalar.dma_start(
                                    yt[ng * 256 + nsl * P:
                                       ng * 256 + (nsl + 1) * P, :],
                                    ev[:])
                        else:
                            # last group: nsl-outer so the nsl=0 banks
                            # stop at the group's midpoint and evict under
                            # nsl=1's matmuls; only one bank chain drains
                            # after the final MM
                            for nsl in range(2):
                                for k in range(KT):
                                    lhsT = wt[:, k * 256 + nsl * P:
                                              k * 256 + (nsl + 1) * P]
                                    for mc in range(2):
                                        nc.tensor.matmul(
                                            banks[(nsl, mc)][:],
                                            lhsT,
                                            xtiles[k][
                                                :, mc * 512:(mc + 1) * 512],
                                            start=(k == 0),
                                            stop=(k == KT - 1),
                                        )
                                for mc in range(2):
                                    ev = evpool.tile(
                                        [P, 512], mybir.dt.float32,
                                        name="evs", tag="evs")
                                    nc.vector.tensor_copy(
                                        out=ev[:], in_=banks[(nsl, mc)][:])
                                    nc.scalar.dma_start(
                                        yt[ng * 256 + nsl * P:
                                           ng * 256 + (nsl + 1) * P,
                                           mc * 512:(mc + 1) * 512],
                                        ev[:])
    nc.compile()
    _NC_CACHE[repeats] = nc
    return nc


def prepare_in_maps(x: np.ndarray, weight: np.ndarray):
    """Host-side prep: bf16 x^T shards; W^T pre-tiled to [16, 128, 8192]
    (one contiguous slab per output group; values 0..255 exact in bf16)."""
    bf16 = ml_dtypes.bfloat16
    x2 = np.ascontiguousarray(np.asarray(x).reshape(TOKENS, D_IN))
    kxm_full = np.ascontiguousarray(x2.astype(bf16).T)  # [D_IN, TOKENS]

    wt = np.asarray(weight).astype(np.float32).astype(bf16).T  # [D_IN, D_OUT]
    wt = wt.reshape(32, 128, NG, 256).transpose(2, 1, 0, 3)
    kxns = np.ascontiguousarray(wt.reshape(NG, 128, 32 * 256))

    in_maps = []
    for c in range(N_CORES):
        kxm_c = np.ascontiguousarray(
            kxm_full[:, c * T_SHARD:(c + 1) * T_SHARD])
        in_maps.append({"kxm": kxm_c, "kxns": kxns})
    return in_maps


def gather_output(results):
    y = np.concatenate(
        [np.ascontiguousarray(results[c]["yt"].T) for c in range(N_CORES)],
        axis=0)
    return y.reshape(4, 2048, D_OUT).astype(np.float32, copy=False)


def kernel(x: np.ndarray, weight: np.ndarray) -> np.ndarray:
    from concourse.bass_utils import run_bass_kernel_spmd

    nc = build_nc()
    in_maps = prepare_in_maps(x, weight)
    res = run_bass_kernel_spmd(nc, in_maps, core_ids=list(range(N_CORES)))
    return gather_output(res.results)


# revision 16
# speedup vs baseline: 1.0255x; 1.0255x over previous
"""Trainium2 Bass kernel for BitLinear: y[b,s,o] = sum_d x[b,s,d] * w[o,d].

x: [4, 2048, 4096] f32, weight: [4096, 4096] int32 (values 0..255), y f32.

Strategy:
- Data-parallel over tokens: 8192 tokens -> 8 cores x 1024 tokens.
- Precision: SINGLE bf16 pass. Weight values 0..255 are exact in bf16
  (8-bit mantissa), so the only error is x's bf16 quantization (~2^-9
  rel): max rel err ~1.9e-3 against the 2e-2 gate, at half the matmul
  work of a hi/lo split. (fp8 DoubleRow was measured at 2x FLOP rate =
  157 TF/s, but the accuracy-required 3-term scheme needs 1.5x the
  bf16 single-pass time -> strictly worse.)
- W-stationary: yt[n, m] = W^T[k, n]^T @ X^T[k, m]; each 128-col
  stationary load feeds 2 moving M=512 matmuls (the two 512-token PSUM
  banks). 2048 MMs/core -> ~437 us back-to-back at 2.4 GHz; measured
  PE occupancy ~96% in TimelineSim.
- W is host-pre-tiled to [16, 128, 8192] so each output group's
  weights arrive in ONE contiguous 2MB DMA (16 weight DMAs per pass,
  not 512 -- per-DMA dispatch overhead dominated the un-batched
  version), double-buffered under the previous group's compute. On the
  first pass, groups 0-1 load W in graduated chunks (128KB first) so
  the PE starts ~1.5 us in rather than waiting for a full slab while
  the 8MB x stream owns most of the HBM window.
- X^T shard (8 MB bf16) streams in during group 0 and stays resident.
- Evictions: per 128-row slice, 2 PSUM banks -> one [128,1024] SBUF
  tile -> one DMA (64 vector copies + 32 output DMAs per pass).
- Host gathers per-core yt [4096, 1024] f32, transposes, concatenates.
"""

import sys

for _p in ("/opt/trn_rl_repo", "/root/.axon_site/_ro/trn_rl_repo"):
    if _p not in sys.path:
        sys.path.append(_p)

import numpy as np
import ml_dtypes

N_CORES = 8
TOKENS = 8192
D_IN = 4096
D_OUT = 4096
T_SHARD = TOKENS // N_CORES  # 1024
NG = D_OUT // 256  # 16 output groups

_NC_CACHE = {}


def build_nc(repeats: int = 1):
    """Build (and cache) the Bass program.

    repeats > 1 re-emits the compute body (used only for slope-based HW
    timing; identical output)."""
    if repeats in _NC_CACHE:
        return _NC_CACHE[repeats]

    import concourse.mybir as mybir
    import concourse.tile as tile
    from concourse import bacc

    P = 128
    KT = D_IN // P  # 32
    nc = bacc.Bacc(None, target_bir_lowering=False)
    with tile.TileContext(nc) as tc:
        with tc.tile_pool(name="dram", bufs=1, space="DRAM") as dram:
            kxm = dram.tile([D_IN, T_SHARD], mybir.dt.bfloat16,
                            kind="ExternalInput", name="kxm", uniquify=False)
            kxns = dram.tile([NG, P, KT * 256], mybir.dt.bfloat16,
                             kind="ExternalInput", name="kxns", uniquify=False)
            yt = dram.tile([D_OUT, T_SHARD], mybir.dt.float32,
                           kind="ExternalOutput", name="yt", uniquify=False)
            kxm3 = kxm[:].rearrange("(ko p) m -> p ko m", p=P)  # [128,32,1024]
            with tc.tile_pool(name="xpool", bufs=32) as xpool, \
                 tc.tile_pool(name="wpool", bufs=2) as wpool, \
                 tc.tile_pool(name="pspool", bufs=2, space="PSUM") as pspool, \
                 tc.tile_pool(name="evpool", bufs=4) as evpool:
                xtiles = [None] * KT
                first = True
                for rep in range(repeats):
                    for ng in range(NG):
                        wt = wpool.tile([P, KT * 256], mybir.dt.bfloat16,
                                        name="wt", tag="wt")
                        if rep == 0 and ng <= 1:
                            # cold start: groups 0-1 load W in chunks so
                            # their first matmuls wait on the first 128KB,
                            # not the full 2MB slab (the x stream owns most
                            # of the HBM window during group 0)
                            bounds = [0, 2, 8, 16, 24, 32]
                            for q in range(len(bounds) - 1):
                                nc.sync.dma_start(
                                    wt[:, bounds[q] * 256:bounds[q + 1] * 256],
                                    kxns[ng][:, bounds[q] * 256:
                                             bounds[q + 1] * 256])
                        else:
                            nc.sync.dma_start(wt[:], kxns[ng])
                        banks = {}
                        for nsl in range(2):
                            for mc in range(2):
                                banks[(nsl, mc)] = pspool.tile(
                                    [P, 512], mybir.dt.float32,
                                    name=f"bank_{nsl}_{mc}",
                                    tag=f"bank_{nsl}_{mc}")
                        if ng < NG - 1:
                            for k in range(KT):
                                if first:
                                    # JIT x tiles: X streaming hides under
                                    # group-0 compute and stays resident;
                                    # Activation-engine DGE queue so x
                                    # streams concurrently with W on SP's
                                    xt = xpool.tile([P, T_SHARD],
                                                    mybir.dt.bfloat16,
                                                    name="xt", tag="xt")
                                    nc.scalar.dma_start(xt[:], kxm3[:, k])
                                    xtiles[k] = xt
                                for nsl in range(2):
                                    lhsT = wt[:, k * 256 + nsl * P:
                                              k * 256 + (nsl + 1) * P]
                                    for mc in range(2):
                                        nc.tensor.matmul(
                                            banks[(nsl, mc)][:],
                                            lhsT,
                                            xtiles[k][
                                                :, mc * 512:(mc + 1) * 512],
                                            start=(k == 0),
                                            stop=(k == KT - 1),
                                        )
                            first = False
                            for nsl in range(2):
                                ev = evpool.tile(
                                    [P, T_SHARD], mybir.dt.float32,
                                    name="ev", tag="ev")
                                for mc in range(2):
                                    nc.vector.tensor_copy(
                                        out=ev[:, mc * 512:(mc + 1) * 512],
                                        in_=banks[(nsl, mc)][:])
                                nc.scalar.dma_start(
                                    yt[ng * 256 + nsl * P:
                                       ng * 256 + (nsl + 1) * P, :],
                                    ev[:])
                        else:
                            # last group: nsl-outer so the nsl=0 banks
                            # stop at the group's midpoint and evict under
                            # nsl=1's matmuls; only one bank chain drains
                            # after the final MM
                            for nsl in range(2):
                                for k in range(KT):
                                    lhsT = wt[:, k * 256 + nsl * P:
                                              k * 256 + (nsl + 1) * P]
                                    for mc in range(2):
                                        nc.tensor.matmul(
                                            banks[(nsl, mc)][:],
                                            lhsT,
                                            xtiles[k][
                                                :, mc * 512:(mc + 1) * 512],
                                            start=(k == 0),
                                            stop=(k == KT - 1),
                                        )
                                for mc in range(2):
                                    ev = evpool.tile(
                                        [P, 512], mybir.dt.float32,
                                        name="evs", tag="evs")
                                    nc.vector.tensor_copy(
                                        out=ev[:], in_=banks[(nsl, mc)][:])
                                    nc.scalar.dma_start(
                                        yt[ng * 256 + nsl * P:
                                           ng * 256 + (nsl + 1) * P,
                                           mc * 512:(mc + 1) * 512],
                                        ev[:])
    nc.compile()
    _NC_CACHE[repeats] = nc
    return nc


def prepare_in_maps(x: np.ndarray, weight: np.ndarray):
    """Host-side prep: bf16 x^T shards; W^T pre-tiled to [16, 128, 8192]
    (one contiguous slab per output group; values 0..255 exact in bf16)."""
    bf16 = ml_dtypes.bfloat16
    x2 = np.ascontiguousarray(np.asarray(x).reshape(TOKENS, D_IN))
    kxm_full = np.ascontiguousarray(x2.astype(bf16).T)  # [D_IN, TOKENS]

    wt = np.asarray(weight).astype(np.float32).astype(bf16).T  # [D_IN, D_OUT]
    wt = wt.reshape(32, 128, NG, 256).transpose(2, 1, 0, 3)
    kxns = np.ascontiguousarray(wt.reshape(NG, 128, 32 * 256))

    in_maps = []
    for c in range(N_CORES):
        kxm_c = np.ascontiguousarray(
            kxm_full[:, c * T_SHARD:(c + 1) * T_SHARD])
        in_maps.append({"kxm": kxm_c, "kxns": kxns})
    return in_maps


def gather_output(results):
    y = np.concatenate(
        [np.ascontiguousarray(results[c]["yt"].T) for c in range(N_CORES)],
        axis=0)
    return y.reshape(4, 2048, D_OUT).astype(np.float32, copy=False)


def kernel(x: np.ndarray, weight: np.ndarray) -> np.ndarray:
    from concourse.bass_utils import run_bass_kernel_spmd

    nc = build_nc()
    in_maps = prepare_in_maps(x, weight)
    res = run_bass_kernel_spmd(nc, in_maps, core_ids=list(range(N_CORES)))
    return gather_output(res.results)


# revision 21
# speedup vs baseline: 1.1338x; 1.1056x over previous
"""Trainium2 Bass kernel for BitLinear: y[b,s,o] = sum_d x[b,s,d] * w[o,d].

x: [4, 2048, 4096] f32, weight: [4096, 4096] int32 (values 0..255), y f32.

Strategy:
- Data-parallel over tokens: 8192 tokens -> 8 cores x 1024 tokens.
- Precision: SINGLE bf16 pass. Weight values 0..255 are exact in bf16
  (8-bit mantissa), so the only error is x's bf16 quantization (~2^-9
  rel): max rel err ~1.9e-3 against the 2e-2 gate, at half the matmul
  work of a hi/lo split. (fp8 DoubleRow was measured at 2x FLOP rate =
  157 TF/s, but the accuracy-required 3-term scheme needs 1.5x the
  bf16 single-pass time -> strictly worse.)
- W-stationary: yt[n, m] = W^T[k, n]^T @ X^T[k, m]; each 128-col
  stationary load feeds 2 moving M=512 matmuls (the two 512-token PSUM
  banks). 2048 MMs/core -> ~437 us back-to-back at 2.4 GHz; measured
  PE occupancy ~96% in TimelineSim.
- W is host-pre-tiled to [16, 128, 8192] so each output group's
  weights arrive in ONE contiguous 2MB DMA (16 weight DMAs per pass,
  not 512 -- per-DMA dispatch overhead dominated the un-batched
  version), double-buffered under the previous group's compute. On the
  first pass, groups 0-1 load W in graduated chunks (128KB first) so
  the PE starts ~1.5 us in rather than waiting for a full slab while
  the 8MB x stream owns most of the HBM window.
- X^T shard (8 MB bf16) streams in during group 0 and stays resident.
- Evictions: per 128-row slice, 2 PSUM banks -> one [128,1024] SBUF
  tile -> one DMA (64 vector copies + 32 output DMAs per pass).
- Host gathers per-core yt [4096, 1024] f32, transposes, concatenates.
"""

import sys

for _p in ("/opt/trn_rl_repo", "/root/.axon_site/_ro/trn_rl_repo"):
    if _p not in sys.path:
        sys.path.append(_p)

import numpy as np
import ml_dtypes

N_CORES = 8
TOKENS = 8192
D_IN = 4096
D_OUT = 4096
T_SHARD = TOKENS // N_CORES  # 1024
NG = D_OUT // 256  # 16 output groups

_NC_CACHE = {}


def build_nc(repeats: int = 1):
    """Build (and cache) the Bass program.

    repeats > 1 re-emits the compute body (used only for slope-based HW
    timing; identical output)."""
    if repeats in _NC_CACHE:
        return _NC_CACHE[repeats]

    import concourse.mybir as mybir
    import concourse.tile as tile
    from concourse import bacc

    P = 128
    KT = D_IN // P  # 32
    nc = bacc.Bacc(None, target_bir_lowering=False)
    with tile.TileContext(nc) as tc:
        with tc.tile_pool(name="dram", bufs=1, space="DRAM") as dram:
            kxm = dram.tile([D_IN, T_SHARD], mybir.dt.bfloat16,
                            kind="ExternalInput", name="kxm", uniquify=False)
            kxns = dram.tile([NG, P, KT * 256], mybir.dt.bfloat16,
                             kind="ExternalInput", name="kxns", uniquify=False)
            yt = dram.tile([D_OUT, T_SHARD], mybir.dt.float32,
                           kind="ExternalOutput", name="yt", uniquify=False)
            kxm3 = kxm[:].rearrange("(ko p) m -> p ko m", p=P)  # [128,32,1024]
            with tc.tile_pool(name="xpool", bufs=32) as xpool, \
                 tc.tile_pool(name="wpool", bufs=2) as wpool, \
                 tc.tile_pool(name="pspool", bufs=2, space="PSUM") as pspool, \
                 tc.tile_pool(name="evpool", bufs=4) as evpool:
                xtiles = [None] * KT
                first = True
                for rep in range(repeats):
                    for ng in range(NG):
                        wt = wpool.tile([P, KT * 256], mybir.dt.bfloat16,
                                        name="wt", tag="wt")
                        if rep == 0 and ng <= 1:
                            # cold start: groups 0-1 load W in chunks so
                            # their first matmuls wait on the first 128KB,
                            # not the full 2MB slab (the x stream owns most
                            # of the HBM window during group 0)
                            bounds = [0, 2, 8, 16, 24, 32]
                            for q in range(len(bounds) - 1):
                                nc.sync.dma_start(
                                    wt[:, bounds[q] * 256:bounds[q + 1] * 256],
                                    kxns[ng][:, bounds[q] * 256:
                                             bounds[q + 1] * 256])
                        else:
                            nc.sync.dma_start(wt[:], kxns[ng])
                        banks = {}
                        for nsl in range(2):
                            for mc in range(2):
                                banks[(nsl, mc)] = pspool.tile(
                                    [P, 512], mybir.dt.float32,
                                    name=f"bank_{nsl}_{mc}",
                                    tag=f"bank_{nsl}_{mc}")
                        if ng < NG - 1:
                            for k in range(KT):
                                if first:
                                    # JIT x tiles: X streaming hides under
                                    # group-0 compute and stays resident;
                                    # Activation-engine DGE queue so x
                                    # streams concurrently with W on SP's
                                    xt = xpool.tile([P, T_SHARD],
                                                    mybir.dt.bfloat16,
                                                    name="xt", tag="xt")
                                    nc.scalar.dma_start(xt[:], kxm3[:, k])
                                    xtiles[k] = xt
                                for nsl in range(2):
                                    lhsT = wt[:, k * 256 + nsl * P:
                                              k * 256 + (nsl + 1) * P]
                                    for mc in range(2):
                                        nc.tensor.matmul(
                                            banks[(nsl, mc)][:],
                                            lhsT,
                                            xtiles[k][
                                                :, mc * 512:(mc + 1) * 512],
                                            start=(k == 0),
                                            stop=(k == KT - 1),
                                        )
                            first = False
                            for nsl in range(2):
                                ev = evpool.tile(
                                    [P, T_SHARD], mybir.dt.float32,
                                    name="ev", tag="ev")
                                for mc in range(2):
                                    nc.vector.tensor_copy(
                                        out=ev[:, mc * 512:(mc + 1) * 512],
                                        in_=banks[(nsl, mc)][:])
                                nc.scalar.dma_start(
                                    yt[ng * 256 + nsl * P:
                                       ng * 256 + (nsl + 1) * P, :],
                                    ev[:])
                        else:
                            # last group: nsl-outer so the nsl=0 banks
                            # stop at the group's midpoint and evict under
                            # nsl=1's matmuls; only one bank chain drains
                            # after the final MM
                            for nsl in range(2):
                                for k in range(KT):
                                    lhsT = wt[:, k * 256 + nsl * P:
                                              k * 256 + (nsl + 1) * P]
                                    for mc in range(2):
                                        nc.tensor.matmul(
                                            banks[(nsl, mc)][:],
                                            lhsT,
                                            xtiles[k][
                                                :, mc * 512:(mc + 1) * 512],
                                            start=(k == 0),
                                            stop=(k == KT - 1),
                                        )
                                for mc in range(2):
                                    ev = evpool.tile(
                                        [P, 512], mybir.dt.float32,
                                        name="evs", tag="evs")
                                    nc.vector.tensor_copy(
                                        out=ev[:], in_=banks[(nsl, mc)][:])
                                    nc.scalar.dma_start(
                                        yt[ng * 256 + nsl * P:
                                           ng * 256 + (nsl + 1) * P,
                                           mc * 512:(mc + 1) * 512],
                                        ev[:])
    nc.compile()
    _NC_CACHE[repeats] = nc
    return nc


def prepare_in_maps(x: np.ndarray, weight: np.ndarray):
    """Host-side prep: bf16 x^T shards; W^T pre-tiled to [16, 128, 8192]
    (one contiguous slab per output group; values 0..255 exact in bf16)."""
    bf16 = ml_dtypes.bfloat16
    x2 = np.ascontiguousarray(np.asarray(x).reshape(TOKENS, D_IN))
    kxm_full = np.ascontiguousarray(x2.astype(bf16).T)  # [D_IN, TOKENS]

    wt = np.asarray(weight).astype(np.float32).astype(bf16).T  # [D_IN, D_OUT]
    wt = wt.reshape(32, 128, NG, 256).transpose(2, 1, 0, 3)
    kxns = np.ascontiguousarray(wt.reshape(NG, 128, 32 * 256))

    in_maps = []
    for c in range(N_CORES):
        kxm_c = np.ascontiguousarray(
            kxm_full[:, c * T_SHARD:(c + 1) * T_SHARD])
        in_maps.append({"kxm": kxm_c, "kxns": kxns})
    return in_maps


def gather_output(results):
    y = np.concatenate(
        [np.ascontiguousarray(results[c]["yt"].T) for c in range(N_CORES)],
        axis=0)
    return y.reshape(4, 2048, D_OUT).astype(np.float32, copy=False)


def kernel(x: np.ndarray, weight: np.ndarray) -> np.ndarray:
    from concourse.bass_utils import run_bass_kernel_spmd

    nc = build_nc()
    in_maps = prepare_in_maps(x, weight)
    res = run_bass_kernel_spmd(nc, in_maps, core_ids=list(range(N_CORES)))
    return gather_output(res.results)
